# revision 7
# baseline (speedup 1.0000x reference)
"""MiMoV2 sliding-window attention (S=2048, H=32, KVH=8, D=64, WIN=512) on 8
Trainium2 NeuronCores, tensor-parallel across heads (4 q heads / 1 kv head per
core), host-side reduce of the w_o partial sums.

Per-core device kernel:
  hiddenT [HID, S] streamed as fp32r k-tiles; qT/kT/vT produced in transposed
  [head_dim, seq] form directly by the projection matmuls; neox rope applied
  via SBUF->SBUF half-swap DMAs plus sign-folded trig tables (q additionally
  pre-scaled by D^-0.5); scoresT [j, i] = kT.T @ qT per 128-key j-tile; exp on
  ACT (psum->sbuf, two j-tiles per call); sliding-window mask via one gpsimd
  affine_select per tile; PV with an appended ones-column producing the
  softmax denominator for free; denominator + attention-sink + reciprocal on
  DVE; partition-broadcast of 1/denom via a K=2 indicator matmul; w_o partial
  per s-tile, summed across cores on the host.
"""
import os
import numpy as np

S = 2048
HID = 2048
H = 32
KVH = 8
D = 64
WIN = 512
THETA = 1e6
NCORES = 8
HPC = H // NCORES        # q heads per core = 4
QCOLS = HPC * D          # 256

_COMPILED = None


def _build_kernel():
    import concourse.bass as bass
    import concourse.tile as tile
    from concourse import bacc, mybir

    FP32 = mybir.dt.float32
    FP32R = mybir.dt.float32r
    AF = mybir.ActivationFunctionType

    nc = bacc.Bacc("TRN2", target_bir_lowering=False, debug=False,
                   num_devices=NCORES)

    hT = nc.dram_tensor("hT", [HID, S], FP32R, kind="ExternalInput")
    wq = nc.dram_tensor("wq", [HID, QCOLS], FP32R, kind="ExternalInput")
    wkv = nc.dram_tensor("wkv", [HID, 128], FP32R, kind="ExternalInput")
    wo = nc.dram_tensor("wo", [QCOLS, HID], FP32R, kind="ExternalInput")
    cosq = nc.dram_tensor("cosq", [128, S], FP32R, kind="ExternalInput")
    sinq = nc.dram_tensor("sinq", [128, S], FP32R, kind="ExternalInput")
    es_flat = nc.dram_tensor("es_flat", [1, HPC], FP32, kind="ExternalInput")
    ones_col = nc.dram_tensor("ones_col", [128, 1], FP32R, kind="ExternalInput")
    ones_row = nc.dram_tensor("ones_row", [1, 64], FP32R, kind="ExternalInput")
    idn64 = nc.dram_tensor("idn64", [64, 64], FP32, kind="ExternalInput")

    out_part = nc.dram_tensor("out_part", [S, HID], FP32, kind="ExternalOutput")
    kT_out = nc.dram_tensor("kT_out", [64, S], FP32R, kind="ExternalOutput")
    vT_out = nc.dram_tensor("vT_out", [64, S], FP32, kind="ExternalOutput")

    NJ = S // 128
    NIC = S // 512

    with tile.TileContext(nc) as tc:
        with (
            tc.tile_pool(name="const", bufs=1) as cpool,
            tc.tile_pool(name="wpool", bufs=1) as wpool,
            tc.tile_pool(name="hpool", bufs=2) as hpool,
            tc.tile_pool(name="qk", bufs=1) as qkpool,
            tc.tile_pool(name="ppool", bufs=3) as ppool,
            tc.tile_pool(name="npool", bufs=1) as npool,
            tc.tile_pool(name="opool", bufs=2) as opool,
        ):
            # ---- constants ----
            cosq_sb = cpool.tile([128, S], FP32R)
            nc.sync.dma_start(cosq_sb[:], cosq[:])
            sinq_sb = cpool.tile([128, S], FP32R)
            nc.sync.dma_start(sinq_sb[:], sinq[:])
            es_sb = cpool.tile([1, HPC], FP32)
            nc.sync.dma_start(es_sb[:], es_flat[:])
            ones_sb = cpool.tile([128, 1], FP32R)
            nc.sync.dma_start(ones_sb[:], ones_col[:])
            onesr_sb = cpool.tile([1, 64], FP32R)
            nc.sync.dma_start(onesr_sb[:], ones_row[:])
            idn_sb = cpool.tile([64, 64], FP32)
            nc.sync.dma_start(idn_sb[:], idn64[:])

            # ---- weights ----
            wq_sb = wpool.tile([128, 16 * QCOLS], FP32R)
            nc.sync.dma_start(
                wq_sb[:].rearrange("p (k m) -> p k m", k=16),
                wq.rearrange("(k p) m -> p k m", p=128),
            )
            wkv_sb = wpool.tile([128, 16 * 128], FP32R)
            nc.sync.dma_start(
                wkv_sb[:].rearrange("p (k m) -> p k m", k=16),
                wkv.rearrange("(k p) m -> p k m", p=128),
            )
            wo_sb = []
            for k in range(2):
                w = wpool.tile([128, HID], FP32R, tag=f"wo{k}", name=f"wo_sb{k}")
                nc.sync.dma_start(w[:], wo[k * 128:(k + 1) * 128, :])
                wo_sb.append(w)

            # ---- phase 1: projections (qT/kT/vT in [d, s] form) ----
            # qraw pair tiles later reused as normalized attn-out tiles.
            qraw = [qkpool.tile([128, S], FP32R, tag=f"qraw{t}", name=f"qraw{t}")
                    for t in range(2)]
            kraw = qkpool.tile([64, S], FP32R)
            vT = qkpool.tile([64, S], FP32)

            ps_proj_cm = tc.tile_pool(name="ps_proj", bufs=1, space="PSUM")
            ps_proj = ps_proj_cm.__enter__()
            for sh in range(2):
                sl = slice(sh * 1024, (sh + 1) * 1024)
                q_ps = [ps_proj.tile([128, 1024], FP32, tag=f"q_ps{m}",
                                     name=f"q_ps{m}_{sh}") for m in range(2)]
                kv_ps = ps_proj.tile([128, 1024], FP32, tag="kv_ps",
                                     name=f"kv_ps_{sh}")
                for k in range(16):
                    ht = hpool.tile([128, 1024], FP32R, tag="ht", name=f"ht{sh}_{k}")
                    nc.sync.dma_start(ht[:], hT[k * 128:(k + 1) * 128, sl])
                    st, sp = (k == 0), (k == 15)
                    for m in range(2):
                        for n in range(2):
                            nc.tensor.matmul(
                                q_ps[m][:, n * 512:(n + 1) * 512],
                                wq_sb[:, k * QCOLS + m * 128:k * QCOLS + (m + 1) * 128],
                                ht[:, n * 512:(n + 1) * 512],
                                start=st, stop=sp,
                            )
                    for n in range(2):
                        nc.tensor.matmul(
                            kv_ps[:, n * 512:(n + 1) * 512],
                            wkv_sb[:, k * 128:(k + 1) * 128],
                            ht[:, n * 512:(n + 1) * 512],
                            start=st, stop=sp,
                        )
                for m in range(2):
                    nc.scalar.copy(qraw[m][0:64, sl], q_ps[m][0:64, :])
                    nc.vector.tensor_copy(qraw[m][64:128, sl], q_ps[m][64:128, :])
                nc.scalar.copy(kraw[0:64, sl], kv_ps[0:64, :])
                nc.vector.tensor_copy(vT[0:64, sl], kv_ps[64:128, :])
            ps_proj_cm.__exit__(None, None, None)

            # ---- phase 2: rope ----
            qTr = [qkpool.tile([128, S], FP32R, tag=f"qTr{t}", name=f"qTr{t}")
                   for t in range(2)]
            kTr = qkpool.tile([64, S], FP32R)

            for t in range(2):
                jb = npool.tile([128, S], FP32R, tag="jbuf", name=f"jbuf{t}")
                for blk in range(2):
                    b0 = blk * 64
                    nc.sync.dma_start(jb[b0:b0 + 32, :], qraw[t][b0 + 32:b0 + 64, :])
                    nc.sync.dma_start(jb[b0 + 32:b0 + 64, :], qraw[t][b0:b0 + 32, :])
                nc.vector.tensor_mul(qTr[t][:], qraw[t][:], cosq_sb[:])
                nc.vector.tensor_mul(jb[:], jb[:], sinq_sb[:])
                nc.vector.tensor_add(qTr[t][:], qTr[t][:], jb[:])
            jbk = npool.tile([64, S], FP32R)
            nc.sync.dma_start(jbk[0:32, :], kraw[32:64, :])
            nc.sync.dma_start(jbk[32:64, :], kraw[0:32, :])
            nc.vector.tensor_mul(kTr[:], kraw[:], cosq_sb[0:64, :])
            nc.vector.tensor_mul(jbk[:], jbk[:], sinq_sb[0:64, :])
            nc.vector.tensor_add(kTr[:], kTr[:], jbk[:])

            # odd heads into base-0 tiles (matmul rhs must share lhsT's base)
            qodd = [qkpool.tile([64, S], FP32R, tag=f"qodd{t}", name=f"qodd{t}")
                    for t in range(2)]
            for t in range(2):
                nc.vector.tensor_copy(qodd[t][:], qTr[t][64:128, :])

            nc.sync.dma_start(kT_out[:], kTr[:])
            nc.sync.dma_start(vT_out[:], vT[:])

            # ---- phase 3: v65 tiles [j, d | 1] via PE transpose ----
            ps_tp_cm = tc.tile_pool(name="ps_tp", bufs=2, space="PSUM")
            ps_tp = ps_tp_cm.__enter__()
            v65_all = qkpool.tile([128, NJ * 65], FP32R)
            for jt in range(NJ):
                tp = ps_tp.tile([128, 64], FP32, tag="tp", name=f"tp{jt}")
                nc.tensor.transpose(
                    tp[:], vT[:, jt * 128:(jt + 1) * 128], idn_sb[:]
                )
                nc.vector.tensor_copy(v65_all[:, jt * 65:jt * 65 + 64], tp[:])
                nc.vector.tensor_copy(v65_all[:, jt * 65 + 64:jt * 65 + 65],
                                      ones_sb[:])
            ps_tp_cm.__exit__(None, None, None)

            # ---- phase 4: attention + normalize (attn written into qraw) ----
            attn = qraw

            ps_sc_cm = tc.tile_pool(name="ps_sc", bufs=2, space="PSUM")
            ps_sc = ps_sc_cm.__enter__()
            ps_pv_cm = tc.tile_pool(name="ps_pv", bufs=1, space="PSUM")
            ps_pv = ps_pv_cm.__enter__()
            ps_bc_cm = tc.tile_pool(name="ps_bc", bufs=1, space="PSUM")
            ps_bc = ps_bc_cm.__enter__()
            for ic in range(NIC):
                i0 = ic * 512
                jlo = max(0, i0 - 511) // 128
                jhi = (i0 + 511) // 128
                jts = list(range(jlo, jhi + 1))
                groups = [jts[g:g + 2] for g in range(0, len(jts), 2)]
                for t in range(2):
                    pv_ps = []
                    for hh in range(2):
                        h = 2 * t + hh
                        pv = ps_pv.tile([65, 512], FP32, tag=f"pv{hh}",
                                        name=f"pv{h}_{ic}")
                        pv_ps.append(pv)
                        if hh == 0:
                            qsrc = qTr[t][0:64, i0:i0 + 512]
                        else:
                            qsrc = qodd[t][:, i0:i0 + 512]
                        nmm = 0
                        for grp in groups:
                            sc = ps_sc.tile([128, 1024], FP32, tag="sc",
                                            name=f"sc{h}_{ic}_{grp[0]}")
                            for gi, jt in enumerate(grp):
                                nc.tensor.matmul(
                                    sc[:, gi * 512:(gi + 1) * 512],
                                    kTr[:, jt * 128:(jt + 1) * 128], qsrc,
                                    start=True, stop=True,
                                )
                            p = ppool.tile([128, 1024], FP32R, tag="p",
                                           name=f"p{h}_{ic}_{grp[0]}")
                            nc.scalar.activation(
                                p[:, 0:len(grp) * 512], sc[:, 0:len(grp) * 512],
                                AF.Exp,
                            )
                            for gi, jt in enumerate(grp):
                                delta = i0 - jt * 128
                                psl = p[:, gi * 512:(gi + 1) * 512]
                                if delta <= 0:
                                    nc.gpsimd.affine_select(
                                        psl, psl, pattern=[[1, 512]],
                                        compare_op=mybir.AluOpType.is_ge,
                                        fill=0.0, base=delta,
                                        channel_multiplier=-1,
                                    )
                                else:
                                    # keep iff delta + a - b <= 511, rewritten
                                    # as (511 - delta) + b - a >= 0
                                    nc.gpsimd.affine_select(
                                        psl, psl, pattern=[[-1, 512]],
                                        compare_op=mybir.AluOpType.is_ge,
                                        fill=0.0, base=511 - delta,
                                        channel_multiplier=1,
                                    )
                                nc.tensor.matmul(
                                    pv[:], v65_all[:, jt * 65:(jt + 1) * 65], psl,
                                    start=(nmm == 0), stop=(nmm == len(jts) - 1),
                                )
                                nmm += 1
                    # normalize the pair (everything at base partition 0)
                    bc_sb = npool.tile([128, 512], FP32, tag="bc_sb",
                                       name=f"bcsb{t}_{ic}")
                    araw = npool.tile([128, 512], FP32, tag="araw",
                                      name=f"araw{t}_{ic}")
                    for hh in range(2):
                        h = 2 * t + hh
                        den = npool.tile([1, 512], FP32, tag=f"den{hh}",
                                         name=f"den{h}_{ic}")
                        nc.vector.tensor_copy(den[:], pv_ps[hh][64:65, :])
                        nc.vector.tensor_scalar_add(den[:], den[:],
                                                    es_sb[0:1, h:h + 1])
                        rec = npool.tile([1, 512], FP32R, tag=f"rec{hh}",
                                         name=f"rec{h}_{ic}")
                        with nc.allow_low_precision(reason="recip for bcast"):
                            nc.vector.reciprocal(rec[:], den[:])
                        bc = ps_bc.tile([64, 512], FP32, tag=f"bc{hh}",
                                        name=f"bc{h}_{ic}")
                        nc.tensor.matmul(bc[:], onesr_sb[:], rec[:],
                                         start=True, stop=True)
                        nc.vector.tensor_copy(
                            bc_sb[hh * 64:(hh + 1) * 64, :], bc[:])
                        if hh == 0:
                            nc.scalar.copy(araw[0:64, :], pv_ps[0][0:64, :])
                        else:
                            nc.vector.tensor_copy(araw[64:128, :],
                                                  pv_ps[1][0:64, :])
                    nc.vector.tensor_mul(attn[t][:, i0:i0 + 512], araw[:],
                                         bc_sb[:])
            ps_bc_cm.__exit__(None, None, None)
            ps_pv_cm.__exit__(None, None, None)
            ps_sc_cm.__exit__(None, None, None)

            # ---- phase 5: w_o partial ----
            ps_wo_cm = tc.tile_pool(name="ps_wo", bufs=2, space="PSUM")
            ps_wo = ps_wo_cm.__enter__()
            for st in range(16):
                s0 = st * 128
                woo = ps_wo.tile([128, 2048], FP32, tag="woo", name=f"woo{st}")
                for n in range(4):
                    for k in range(2):
                        nc.tensor.matmul(
                            woo[:, n * 512:(n + 1) * 512],
                            attn[k][:, s0:s0 + 128],
                            wo_sb[k][:, n * 512:(n + 1) * 512],
                            start=(k == 0), stop=(k == 1),
                        )
                osb = opool.tile([128, 2048], FP32, tag="osb", name=f"osb{st}")
                nc.scalar.copy(osb[:, 0:1024], woo[:, 0:1024])
                nc.vector.tensor_copy(osb[:, 1024:2048], woo[:, 1024:2048])
                nc.sync.dma_start(out_part[s0:s0 + 128, :], osb[:])
            ps_wo_cm.__exit__(None, None, None)

    nc.compile()
    return nc


def _get_compiled():
    global _COMPILED
    if _COMPILED is None:
        _COMPILED = _build_kernel()
    return _COMPILED


def _host_inputs(positions, hidden_states, w_q, w_k, w_v, w_o, sink):
    positions = np.asarray(positions)
    hidden_states = np.asarray(hidden_states, dtype=np.float32)
    w_q = np.asarray(w_q, dtype=np.float32)
    w_k = np.asarray(w_k, dtype=np.float32)
    w_v = np.asarray(w_v, dtype=np.float32)
    w_o = np.asarray(w_o, dtype=np.float32)
    sink = np.asarray(sink, dtype=np.float32)

    hT = np.ascontiguousarray(hidden_states.T)

    half = D // 2
    inv_freq = 1.0 / (THETA ** (np.arange(half, dtype=np.float32) / half))
    ang = positions.astype(np.float32)[:, None] * inv_freq[None, :]
    cos = np.cos(ang).T.astype(np.float32)          # [32, S]
    sin = np.sin(ang).T.astype(np.float32)

    cos2 = np.concatenate([cos, cos], 0)            # [64, S]
    sin2s = np.concatenate([-sin, sin], 0)          # signed
    cosq_t = np.ascontiguousarray(np.concatenate([cos2, cos2], 0))
    sinq_t = np.ascontiguousarray(np.concatenate([sin2s, sin2s], 0))

    idn64 = np.eye(64, dtype=np.float32)
    ones_col = np.ones((128, 1), np.float32)

    es = np.exp(sink).astype(np.float32)

    in_maps = []
    for c in range(NCORES):
        wq_c = np.ascontiguousarray(w_q[:, c * QCOLS:(c + 1) * QCOLS]) * np.float32(D ** -0.5)
        wkv_c = np.ascontiguousarray(
            np.concatenate([w_k[:, c * D:(c + 1) * D],
                            w_v[:, c * D:(c + 1) * D]], axis=1))
        wo_c = np.ascontiguousarray(w_o[c * QCOLS:(c + 1) * QCOLS, :])
        es_p = es[c * HPC:(c + 1) * HPC].reshape(1, HPC).copy()
        in_maps.append({
            "hT": hT, "wq": wq_c, "wkv": wkv_c, "wo": wo_c,
            "cosq": cosq_t, "sinq": sinq_t,
            "es_flat": es_p, "ones_col": ones_col,
            "ones_row": np.ones((1, 64), np.float32),
            "idn64": idn64,
        })
    return in_maps


def kernel(positions, hidden_states, w_q, w_k, w_v, w_o, sink, _res_holder=[]):
    from concourse.bass_utils import run_bass_kernel_spmd

    nc = _get_compiled()
    in_maps = _host_inputs(positions, hidden_states, w_q, w_k, w_v, w_o, sink)
    trace = os.environ.get("KERNEL_TRACE", "0") == "1"
    res = run_bass_kernel_spmd(nc, in_maps, list(range(NCORES)), trace=trace)
    _res_holder.clear()
    _res_holder.append(res)

    output = np.zeros((S, HID), np.float32)
    k_full = np.zeros((S, KVH * D), np.float32)
    v_full = np.zeros((S, KVH * D), np.float32)
    for c in range(NCORES):
        r = res.results[c]
        output += r["out_part"]
        k_full[:, c * D:(c + 1) * D] = r["kT_out"].T
        v_full[:, c * D:(c + 1) * D] = r["vT_out"].T
    kv_fused = np.concatenate([k_full, v_full], axis=1)
    return output, kv_fused


# revision 12
# speedup vs baseline: 1.2814x; 1.2814x over previous
"""MiMoV2 sliding-window attention (S=2048, H=32, KVH=8, D=64, WIN=512) on 8
Trainium2 NeuronCores, tensor-parallel across heads (4 q heads / 1 kv head per
core), host-side reduce of the w_o partial sums.

Per-core device kernel:
  hiddenT [HID, S] streamed as fp32r k-tiles; qT/kT/vT produced in transposed
  [head_dim, seq] form directly by the projection matmuls (D^-0.5 folded into
  w_q on the host); neox rope applied via SBUF->SBUF half-swap DMAs plus
  sign-folded trig tables; scoresT [j, i] = kT.T @ qT per 128-key j-tile; exp
  on ACT (psum->sbuf, two j-tiles per call); sliding-window mask via one
  gpsimd affine_select per tile; PV with an appended ones-column producing the
  softmax denominator for free; denominators for one i-chunk are gathered,
  DMA-reshaped to [32, 64] so the (slow, per-lane) DVE reciprocal runs once
  across 32 lanes, DMA'd back row-per-head, and broadcast across partitions
  with a K=2 indicator matmul straight into PSUM for the normalize multiply;
  w_o partial per s-tile (weights loaded into the released w_q slot), summed
  across cores on the host.
"""
import os
import numpy as np

S = 2048
HID = 2048
H = 32
KVH = 8
D = 64
WIN = 512
THETA = 1e6
NCORES = 8
HPC = H // NCORES        # q heads per core = 4
QCOLS = HPC * D          # 256

_COMPILED = None


def _build_kernel():
    import concourse.bass as bass
    import concourse.tile as tile
    from concourse import bacc, mybir

    FP32 = mybir.dt.float32
    FP32R = mybir.dt.float32r
    AF = mybir.ActivationFunctionType

    nc = bacc.Bacc("TRN2", target_bir_lowering=False, debug=False,
                   num_devices=NCORES)

    hT = nc.dram_tensor("hT", [HID, S], FP32R, kind="ExternalInput")
    wq = nc.dram_tensor("wq", [HID, QCOLS], FP32R, kind="ExternalInput")
    wkv = nc.dram_tensor("wkv", [HID, 128], FP32R, kind="ExternalInput")
    wo = nc.dram_tensor("wo", [QCOLS, HID], FP32R, kind="ExternalInput")
    cosq = nc.dram_tensor("cosq", [128, S], FP32R, kind="ExternalInput")
    sinq = nc.dram_tensor("sinq", [128, S], FP32R, kind="ExternalInput")
    es_sh = nc.dram_tensor("es_sh", [128, 1], FP32, kind="ExternalInput")
    ones_col = nc.dram_tensor("ones_col", [128, 1], FP32R, kind="ExternalInput")
    ind2 = nc.dram_tensor("ind2", [2, 128], FP32R, kind="ExternalInput")
    idn64 = nc.dram_tensor("idn64", [64, 64], FP32, kind="ExternalInput")

    den_dram = nc.dram_tensor("den_dram", [4, 2048], FP32,
                              kind="ExternalOutput")
    rec_dram = nc.dram_tensor("rec_dram", [4, 2048], FP32R,
                              kind="ExternalOutput")
    out_part = nc.dram_tensor("out_part", [S, HID], FP32, kind="ExternalOutput")
    kT_out = nc.dram_tensor("kT_out", [64, S], FP32R, kind="ExternalOutput")
    vT_out = nc.dram_tensor("vT_out", [64, S], FP32, kind="ExternalOutput")

    NJ = S // 128
    NIC = S // 512

    with tile.TileContext(nc) as tc:
        with (
            tc.tile_pool(name="const", bufs=1) as cpool,
            tc.tile_pool(name="wpool", bufs=1) as wpool,
            tc.tile_pool(name="hpool", bufs=4) as hpool,
            tc.tile_pool(name="qk", bufs=1) as qkpool,
            tc.tile_pool(name="ppool", bufs=3) as ppool,
            tc.tile_pool(name="npool", bufs=1) as npool,
            tc.tile_pool(name="opool", bufs=2) as opool,
        ):
            # ---- constants ----
            cosq_sb = cpool.tile([128, S], FP32R)
            nc.sync.dma_start(cosq_sb[:], cosq[:])
            sinq_sb = cpool.tile([128, S], FP32R)
            nc.sync.dma_start(sinq_sb[:], sinq[:])
            es_sb = cpool.tile([128, 1], FP32)
            nc.sync.dma_start(es_sb[:], es_sh[:])
            ones_sb = cpool.tile([128, 1], FP32R)
            nc.sync.dma_start(ones_sb[:], ones_col[:])
            ind2_sb = cpool.tile([2, 128], FP32R)
            nc.sync.dma_start(ind2_sb[:], ind2[:])
            idn_sb = cpool.tile([64, 64], FP32)
            nc.sync.dma_start(idn_sb[:], idn64[:])

            # ---- weights (w_q slot recycled for w_o in phase 5) ----
            wq_sb = wpool.tile([128, 16 * QCOLS], FP32R, tag="wqslot",
                               name="wq_sb")
            nc.sync.dma_start(
                wq_sb[:].rearrange("p (k m) -> p k m", k=16),
                wq.rearrange("(k p) m -> p k m", p=128),
            )
            wkv_sb = wpool.tile([128, 16 * 128], FP32R)
            nc.sync.dma_start(
                wkv_sb[:].rearrange("p (k m) -> p k m", k=16),
                wkv.rearrange("(k p) m -> p k m", p=128),
            )

            # ---- phase 1: projections ----
            qraw = [qkpool.tile([128, S], FP32R, tag=f"qraw{t}", name=f"qraw{t}")
                    for t in range(2)]
            kraw = qkpool.tile([64, S], FP32R)
            vT = qkpool.tile([64, S], FP32)

            ps_proj_cm = tc.tile_pool(name="ps_proj", bufs=1, space="PSUM")
            ps_proj = ps_proj_cm.__enter__()
            for sh in range(2):
                sl = slice(sh * 1024, (sh + 1) * 1024)
                q_ps = [ps_proj.tile([128, 1024], FP32, tag=f"q_ps{m}",
                                     name=f"q_ps{m}_{sh}") for m in range(2)]
                kv_ps = ps_proj.tile([128, 1024], FP32, tag="kv_ps",
                                     name=f"kv_ps_{sh}")
                for k in range(16):
                    ht = hpool.tile([128, 1024], FP32R, tag="ht", name=f"ht{sh}_{k}")
                    nc.sync.dma_start(ht[:], hT[k * 128:(k + 1) * 128, sl])
                    st, sp = (k == 0), (k == 15)
                    for m in range(2):
                        for n in range(2):
                            nc.tensor.matmul(
                                q_ps[m][:, n * 512:(n + 1) * 512],
                                wq_sb[:, k * QCOLS + m * 128:k * QCOLS + (m + 1) * 128],
                                ht[:, n * 512:(n + 1) * 512],
                                start=st, stop=sp,
                            )
                    for n in range(2):
                        nc.tensor.matmul(
                            kv_ps[:, n * 512:(n + 1) * 512],
                            wkv_sb[:, k * 128:(k + 1) * 128],
                            ht[:, n * 512:(n + 1) * 512],
                            start=st, stop=sp,
                        )
                for m in range(2):
                    nc.scalar.copy(qraw[m][0:64, sl], q_ps[m][0:64, :])
                    nc.vector.tensor_copy(qraw[m][64:128, sl], q_ps[m][64:128, :])
                nc.scalar.copy(kraw[0:64, sl], kv_ps[0:64, :])
                nc.vector.tensor_copy(vT[0:64, sl], kv_ps[64:128, :])
            ps_proj_cm.__exit__(None, None, None)

            # ---- phase 2: rope ----
            qTr = [qkpool.tile([128, S], FP32R, tag=f"qTr{t}", name=f"qTr{t}")
                   for t in range(2)]
            kTr = qkpool.tile([64, S], FP32R)

            for t in range(2):
                jb = npool.tile([128, S], FP32R, tag="jbuf", name=f"jbuf{t}")
                for blk in range(2):
                    b0 = blk * 64
                    nc.sync.dma_start(jb[b0:b0 + 32, :], qraw[t][b0 + 32:b0 + 64, :])
                    nc.sync.dma_start(jb[b0 + 32:b0 + 64, :], qraw[t][b0:b0 + 32, :])
                nc.vector.tensor_mul(qTr[t][:], qraw[t][:], cosq_sb[:])
                nc.vector.tensor_mul(jb[:], jb[:], sinq_sb[:])
                nc.vector.tensor_add(qTr[t][:], qTr[t][:], jb[:])
            jbk = npool.tile([64, S], FP32R)
            nc.sync.dma_start(jbk[0:32, :], kraw[32:64, :])
            nc.sync.dma_start(jbk[32:64, :], kraw[0:32, :])
            nc.vector.tensor_mul(kTr[:], kraw[:], cosq_sb[0:64, :])
            nc.vector.tensor_mul(jbk[:], jbk[:], sinq_sb[0:64, :])
            nc.vector.tensor_add(kTr[:], kTr[:], jbk[:])

            qodd = [qkpool.tile([64, S], FP32R, tag=f"qodd{t}", name=f"qodd{t}")
                    for t in range(2)]
            for t in range(2):
                nc.vector.tensor_copy(qodd[t][:], qTr[t][64:128, :])

            nc.sync.dma_start(kT_out[:], kTr[:])
            nc.sync.dma_start(vT_out[:], vT[:])

            # ---- phase 3: v65 (v back to [j, d] + ones column) ----
            ps_tp_cm = tc.tile_pool(name="ps_tp", bufs=2, space="PSUM")
            ps_tp = ps_tp_cm.__enter__()
            v65_all = qkpool.tile([128, NJ * 65], FP32R)
            for jt in range(NJ):
                tp = ps_tp.tile([128, 64], FP32, tag="tp", name=f"tp{jt}")
                nc.tensor.transpose(
                    tp[:], vT[:, jt * 128:(jt + 1) * 128], idn_sb[:]
                )
                nc.vector.tensor_copy(v65_all[:, jt * 65:jt * 65 + 64], tp[:])
                nc.vector.tensor_copy(v65_all[:, jt * 65 + 64:jt * 65 + 65],
                                      ones_sb[:])
            ps_tp_cm.__exit__(None, None, None)

            # ---- phase 4: attention + normalize ----
            attn = qraw

            ps_sc_cm = tc.tile_pool(name="ps_sc", bufs=2, space="PSUM")
            ps_sc = ps_sc_cm.__enter__()
            ps_pv_cm = tc.tile_pool(name="ps_pv", bufs=1, space="PSUM")
            ps_pv = ps_pv_cm.__enter__()
            ps_bc_cm = tc.tile_pool(name="ps_bc", bufs=2, space="PSUM")
            ps_bc = ps_bc_cm.__enter__()
            for ic in range(NIC):
                i0 = ic * 512
                jlo = max(0, i0 - 511) // 128
                jhi = (i0 + 511) // 128
                jts = list(range(jlo, jhi + 1))
                groups = [jts[g:g + 2] for g in range(0, len(jts), 2)]
                den_ic = npool.tile([1, 2048], FP32, tag="den_ic",
                                    name=f"den_{ic}", bufs=2)
                pv_pair = []
                for t in range(2):
                    pv = ps_pv.tile([65, 1024], FP32, tag="pv",
                                    name=f"pv{t}_{ic}")
                    pv_pair.append(pv)
                    for hh in range(2):
                        h = 2 * t + hh
                        if hh == 0:
                            qsrc = qTr[t][0:64, i0:i0 + 512]
                        else:
                            qsrc = qodd[t][:, i0:i0 + 512]
                        pvs = pv[:, hh * 512:(hh + 1) * 512]
                        nmm = 0
                        for grp in groups:
                            sc = ps_sc.tile([128, 1024], FP32, tag="sc",
                                            name=f"sc{h}_{ic}_{grp[0]}")
                            for gi, jt in enumerate(grp):
                                nc.tensor.matmul(
                                    sc[:, gi * 512:(gi + 1) * 512],
                                    kTr[:, jt * 128:(jt + 1) * 128], qsrc,
                                    start=True, stop=True,
                                )
                            p = ppool.tile([128, 1024], FP32R, tag="p",
                                           name=f"p{h}_{ic}_{grp[0]}")
                            nc.scalar.activation(
                                p[:, 0:len(grp) * 512], sc[:, 0:len(grp) * 512],
                                AF.Exp,
                            )
                            for gi, jt in enumerate(grp):
                                delta = i0 - jt * 128
                                psl = p[:, gi * 512:(gi + 1) * 512]
                                if delta <= 0:
                                    nc.gpsimd.affine_select(
                                        psl, psl, pattern=[[1, 512]],
                                        compare_op=mybir.AluOpType.is_ge,
                                        fill=0.0, base=delta,
                                        channel_multiplier=-1,
                                    )
                                else:
                                    nc.gpsimd.affine_select(
                                        psl, psl, pattern=[[-1, 512]],
                                        compare_op=mybir.AluOpType.is_ge,
                                        fill=0.0, base=511 - delta,
                                        channel_multiplier=1,
                                    )
                                nc.tensor.matmul(
                                    pvs, v65_all[:, jt * 65:(jt + 1) * 65], psl,
                                    start=(nmm == 0), stop=(nmm == len(jts) - 1),
                                )
                                nmm += 1
                    nc.vector.tensor_copy(
                        den_ic[0:1, t * 1024:(t + 1) * 1024], pv[64:65, :])
                # reciprocal across 32 lanes: [1, 2048] -> [32, 64] via a
                # DRAM bounce (linear DRAM makes the reshape well-defined)
                nc.sync.dma_start(den_dram[ic:ic + 1, :], den_ic[0:1, :])
                den128 = npool.tile([32, 64], FP32, tag="den128",
                                    name=f"den128_{ic}", bufs=2)
                nc.sync.dma_start(den128[:], den_dram[ic:ic + 1, :].rearrange(
                    "a (p f) -> (a p) f", p=32))
                nc.vector.tensor_scalar_add(den128[:], den128[:], es_sb[0:32, :])
                rec128 = npool.tile([32, 64], FP32R, tag="rec128",
                                    name=f"rec128_{ic}", bufs=2)
                with nc.allow_low_precision(reason="recip feeds bcast matmul"):
                    nc.vector.reciprocal(rec128[:], den128[:])
                # back to row-per-head [2, 2048]: row hh, col block t*512
                nc.sync.dma_start(
                    rec_dram[ic:ic + 1, :].rearrange("a (p f) -> (a p) f", p=32),
                    rec128[:])
                rec2 = npool.tile([2, 2048], FP32R, tag="rec2",
                                  name=f"rec2_{ic}", bufs=2)
                for h in range(4):
                    t, hh = h // 2, h % 2
                    nc.sync.dma_start(
                        rec2[hh:hh + 1, t * 512:(t + 1) * 512],
                        rec_dram[ic:ic + 1, h * 512:(h + 1) * 512],
                    )
                for t in range(2):
                    bc = ps_bc.tile([128, 512], FP32, tag="bc",
                                    name=f"bc{t}_{ic}")
                    nc.tensor.matmul(bc[:], ind2_sb[:],
                                     rec2[:, t * 512:(t + 1) * 512],
                                     start=True, stop=True)
                    araw = npool.tile([128, 512], FP32, tag="araw",
                                      name=f"araw{t}_{ic}", bufs=2)
                    nc.scalar.copy(araw[0:64, :], pv_pair[t][0:64, 0:512])
                    nc.vector.tensor_copy(araw[64:128, :],
                                          pv_pair[t][0:64, 512:1024])
                    nc.vector.tensor_mul(attn[t][:, i0:i0 + 512], araw[:],
                                         bc[:])
            ps_bc_cm.__exit__(None, None, None)
            ps_pv_cm.__exit__(None, None, None)
            ps_sc_cm.__exit__(None, None, None)

            # ---- phase 5: w_o partial (weights reuse the w_q slot) ----
            wo_sb = wpool.tile([128, 16 * QCOLS], FP32R, tag="wqslot",
                               name="wo_sb")
            for k in range(2):
                nc.sync.dma_start(wo_sb[:, k * 2048:(k + 1) * 2048],
                                  wo[k * 128:(k + 1) * 128, :])

            ps_wo_cm = tc.tile_pool(name="ps_wo", bufs=2, space="PSUM")
            ps_wo = ps_wo_cm.__enter__()
            for st in range(16):
                s0 = st * 128
                woo = ps_wo.tile([128, 2048], FP32, tag="woo", name=f"woo{st}")
                for n in range(4):
                    for k in range(2):
                        nc.tensor.matmul(
                            woo[:, n * 512:(n + 1) * 512],
                            attn[k][:, s0:s0 + 128],
                            wo_sb[:, k * 2048 + n * 512:k * 2048 + (n + 1) * 512],
                            start=(k == 0), stop=(k == 1),
                        )
                for half in range(2):
                    osb = opool.tile([128, 1024], FP32, tag="osb",
                                     name=f"osb{st}_{half}")
                    if half == 0:
                        nc.scalar.copy(osb[:], woo[:, 0:1024])
                    else:
                        nc.vector.tensor_copy(osb[:], woo[:, 1024:2048])
                    nc.sync.dma_start(
                        out_part[s0:s0 + 128, half * 1024:(half + 1) * 1024],
                        osb[:])
            ps_wo_cm.__exit__(None, None, None)

    nc.compile()
    return nc


def _get_compiled():
    global _COMPILED
    if _COMPILED is None:
        _COMPILED = _build_kernel()
    return _COMPILED


def _host_inputs(positions, hidden_states, w_q, w_k, w_v, w_o, sink):
    positions = np.asarray(positions)
    hidden_states = np.asarray(hidden_states, dtype=np.float32)
    w_q = np.asarray(w_q, dtype=np.float32)
    w_k = np.asarray(w_k, dtype=np.float32)
    w_v = np.asarray(w_v, dtype=np.float32)
    w_o = np.asarray(w_o, dtype=np.float32)
    sink = np.asarray(sink, dtype=np.float32)

    hT = np.ascontiguousarray(hidden_states.T)

    half = D // 2
    inv_freq = 1.0 / (THETA ** (np.arange(half, dtype=np.float32) / half))
    ang = positions.astype(np.float32)[:, None] * inv_freq[None, :]
    cos = np.cos(ang).T.astype(np.float32)          # [32, S]
    sin = np.sin(ang).T.astype(np.float32)

    cos2 = np.concatenate([cos, cos], 0)            # [64, S]
    sin2s = np.concatenate([-sin, sin], 0)          # signed
    cosq_t = np.ascontiguousarray(np.concatenate([cos2, cos2], 0))
    sinq_t = np.ascontiguousarray(np.concatenate([sin2s, sin2s], 0))

    idn64 = np.eye(64, dtype=np.float32)
    ones_col = np.ones((128, 1), np.float32)
    ind2 = np.zeros((2, 128), np.float32)
    ind2[0, 0:64] = 1.0
    ind2[1, 64:128] = 1.0

    es = np.exp(sink).astype(np.float32)

    in_maps = []
    for c in range(NCORES):
        wq_c = np.ascontiguousarray(
            w_q[:, c * QCOLS:(c + 1) * QCOLS]) * np.float32(D ** -0.5)
        wkv_c = np.ascontiguousarray(
            np.concatenate([w_k[:, c * D:(c + 1) * D],
                            w_v[:, c * D:(c + 1) * D]], axis=1))
        wo_c = np.ascontiguousarray(w_o[c * QCOLS:(c + 1) * QCOLS, :])
        es_v = np.zeros((128, 1), np.float32)
        for h in range(HPC):
            es_v[h * 8:(h + 1) * 8, 0] = es[c * HPC + h]
        in_maps.append({
            "hT": hT, "wq": wq_c, "wkv": wkv_c, "wo": wo_c,
            "cosq": cosq_t, "sinq": sinq_t,
            "es_sh": es_v, "ones_col": ones_col,
            "ind2": ind2, "idn64": idn64,
        })
    return in_maps


def kernel(positions, hidden_states, w_q, w_k, w_v, w_o, sink, _res_holder=[]):
    from concourse.bass_utils import run_bass_kernel_spmd

    nc = _get_compiled()
    in_maps = _host_inputs(positions, hidden_states, w_q, w_k, w_v, w_o, sink)
    trace = os.environ.get("KERNEL_TRACE", "0") == "1"
    res = run_bass_kernel_spmd(nc, in_maps, list(range(NCORES)), trace=trace)
    _res_holder.clear()
    _res_holder.append(res)

    output = np.zeros((S, HID), np.float32)
    k_full = np.zeros((S, KVH * D), np.float32)
    v_full = np.zeros((S, KVH * D), np.float32)
    for c in range(NCORES):
        r = res.results[c]
        output += r["out_part"]
        k_full[:, c * D:(c + 1) * D] = r["kT_out"].T
        v_full[:, c * D:(c + 1) * D] = r["vT_out"].T
    kv_fused = np.concatenate([k_full, v_full], axis=1)
    return output, kv_fused


# revision 13
# speedup vs baseline: 1.3503x; 1.0538x over previous
"""MiMoV2 sliding-window attention (S=2048, H=32, KVH=8, D=64, WIN=512) on 8
Trainium2 NeuronCores, tensor-parallel across heads (4 q heads / 1 kv head per
core), host-side reduce of the w_o partial sums.

Per-core device kernel:
  hiddenT [HID, S] streamed as fp32r k-tiles; qT/kT/vT produced in transposed
  [head_dim, seq] form directly by the projection matmuls (D^-0.5 folded into
  w_q on the host); neox rope applied via SBUF->SBUF half-swap DMAs plus
  sign-folded trig tables; scoresT [j, i] = kT.T @ qT per 128-key j-tile; exp
  on ACT (psum->sbuf, two j-tiles per call); sliding-window mask via one
  gpsimd affine_select per tile; PV with an appended ones-column producing the
  softmax denominator for free; denominators for one i-chunk are gathered,
  DMA-reshaped to [32, 64] so the (slow, per-lane) DVE reciprocal runs once
  across 32 lanes, DMA'd back row-per-head, and broadcast across partitions
  with a K=2 indicator matmul straight into PSUM for the normalize multiply;
  w_o partial per s-tile (weights loaded into the released w_q slot), summed
  across cores on the host.
"""
import os
import numpy as np

S = 2048
HID = 2048
H = 32
KVH = 8
D = 64
WIN = 512
THETA = 1e6
NCORES = 8
HPC = H // NCORES        # q heads per core = 4
QCOLS = HPC * D          # 256

_COMPILED = None


def _build_kernel():
    import concourse.bass as bass
    import concourse.tile as tile
    from concourse import bacc, mybir

    FP32 = mybir.dt.float32
    FP32R = mybir.dt.float32r
    AF = mybir.ActivationFunctionType

    nc = bacc.Bacc("TRN2", target_bir_lowering=False, debug=False,
                   num_devices=NCORES)

    hT = nc.dram_tensor("hT", [HID, S], FP32R, kind="ExternalInput")
    wq = nc.dram_tensor("wq", [HID, QCOLS], FP32R, kind="ExternalInput")
    wkv = nc.dram_tensor("wkv", [HID, 128], FP32R, kind="ExternalInput")
    wo = nc.dram_tensor("wo", [QCOLS, HID], FP32R, kind="ExternalInput")
    cosq = nc.dram_tensor("cosq", [128, S], FP32R, kind="ExternalInput")
    sinq = nc.dram_tensor("sinq", [128, S], FP32R, kind="ExternalInput")
    es_sh = nc.dram_tensor("es_sh", [128, 1], FP32, kind="ExternalInput")
    ones_col = nc.dram_tensor("ones_col", [128, 1], FP32R, kind="ExternalInput")
    ind2 = nc.dram_tensor("ind2", [2, 128], FP32R, kind="ExternalInput")
    idn64 = nc.dram_tensor("idn64", [64, 64], FP32, kind="ExternalInput")

    den_dram = nc.dram_tensor("den_dram", [4, 2048], FP32,
                              kind="ExternalOutput")
    rec_dram = nc.dram_tensor("rec_dram", [4, 2048], FP32R,
                              kind="ExternalOutput")
    out_part = nc.dram_tensor("out_part", [S, HID], FP32, kind="ExternalOutput")
    kT_out = nc.dram_tensor("kT_out", [64, S], FP32R, kind="ExternalOutput")
    vT_out = nc.dram_tensor("vT_out", [64, S], FP32, kind="ExternalOutput")

    NJ = S // 128
    NIC = S // 512

    with tile.TileContext(nc) as tc:
        with (
            tc.tile_pool(name="const", bufs=1) as cpool,
            tc.tile_pool(name="wpool", bufs=1) as wpool,
            tc.tile_pool(name="hpool", bufs=4) as hpool,
            tc.tile_pool(name="qk", bufs=1) as qkpool,
            tc.tile_pool(name="ppool", bufs=3) as ppool,
            tc.tile_pool(name="npool", bufs=1) as npool,
            tc.tile_pool(name="opool", bufs=2) as opool,
        ):
            # ---- weights first (per-k contiguous chunks so the first
            # projection matmuls start as soon as chunk 0 lands) ----
            wq_sb = wpool.tile([128, 16 * QCOLS], FP32R, tag="wqslot",
                               name="wq_sb")
            wkv_sb = wpool.tile([128, 16 * 128], FP32R)
            for k in range(16):
                nc.sync.dma_start(wq_sb[:, k * QCOLS:(k + 1) * QCOLS],
                                  wq[k * 128:(k + 1) * 128, :])
                nc.sync.dma_start(wkv_sb[:, k * 128:(k + 1) * 128],
                                  wkv[k * 128:(k + 1) * 128, :])

            # constants (needed from rope onward; scheduled behind phase 1)
            cosq_sb = cpool.tile([128, S], FP32R)
            nc.sync.dma_start(cosq_sb[:], cosq[:])
            sinq_sb = cpool.tile([128, S], FP32R)
            nc.sync.dma_start(sinq_sb[:], sinq[:])
            es_sb = cpool.tile([128, 1], FP32)
            nc.sync.dma_start(es_sb[:], es_sh[:])
            ones_sb = cpool.tile([128, 1], FP32R)
            nc.sync.dma_start(ones_sb[:], ones_col[:])
            ind2_sb = cpool.tile([2, 128], FP32R)
            nc.sync.dma_start(ind2_sb[:], ind2[:])
            idn_sb = cpool.tile([64, 64], FP32)
            nc.sync.dma_start(idn_sb[:], idn64[:])

            # ---- phase 1: projections ----
            qraw = [qkpool.tile([128, S], FP32R, tag=f"qraw{t}", name=f"qraw{t}")
                    for t in range(2)]
            kraw = qkpool.tile([64, S], FP32R)
            vT = qkpool.tile([64, S], FP32)

            ps_proj_cm = tc.tile_pool(name="ps_proj", bufs=1, space="PSUM")
            ps_proj = ps_proj_cm.__enter__()
            for sh in range(2):
                sl = slice(sh * 1024, (sh + 1) * 1024)
                q_ps = [ps_proj.tile([128, 1024], FP32, tag=f"q_ps{m}",
                                     name=f"q_ps{m}_{sh}") for m in range(2)]
                kv_ps = ps_proj.tile([128, 1024], FP32, tag="kv_ps",
                                     name=f"kv_ps_{sh}")
                for k in range(16):
                    ht = hpool.tile([128, 1024], FP32R, tag="ht", name=f"ht{sh}_{k}")
                    nc.sync.dma_start(ht[:], hT[k * 128:(k + 1) * 128, sl])
                    st, sp = (k == 0), (k == 15)
                    for m in range(2):
                        for n in range(2):
                            nc.tensor.matmul(
                                q_ps[m][:, n * 512:(n + 1) * 512],
                                wq_sb[:, k * QCOLS + m * 128:k * QCOLS + (m + 1) * 128],
                                ht[:, n * 512:(n + 1) * 512],
                                start=st, stop=sp,
                            )
                    for n in range(2):
                        nc.tensor.matmul(
                            kv_ps[:, n * 512:(n + 1) * 512],
                            wkv_sb[:, k * 128:(k + 1) * 128],
                            ht[:, n * 512:(n + 1) * 512],
                            start=st, stop=sp,
                        )
                for m in range(2):
                    nc.scalar.copy(qraw[m][0:64, sl], q_ps[m][0:64, :])
                    nc.vector.tensor_copy(qraw[m][64:128, sl], q_ps[m][64:128, :])
                nc.scalar.copy(kraw[0:64, sl], kv_ps[0:64, :])
                nc.vector.tensor_copy(vT[0:64, sl], kv_ps[64:128, :])
            ps_proj_cm.__exit__(None, None, None)

            # ---- phase 2: rope ----
            qTr = [qkpool.tile([128, S], FP32R, tag=f"qTr{t}", name=f"qTr{t}")
                   for t in range(2)]
            kTr = qkpool.tile([64, S], FP32R)

            for t in range(2):
                jb = npool.tile([128, S], FP32R, tag="jbuf", name=f"jbuf{t}")
                for blk in range(2):
                    b0 = blk * 64
                    nc.sync.dma_start(jb[b0:b0 + 32, :], qraw[t][b0 + 32:b0 + 64, :])
                    nc.sync.dma_start(jb[b0 + 32:b0 + 64, :], qraw[t][b0:b0 + 32, :])
                nc.vector.tensor_mul(qTr[t][:], qraw[t][:], cosq_sb[:])
                nc.vector.tensor_mul(jb[:], jb[:], sinq_sb[:])
                nc.vector.tensor_add(qTr[t][:], qTr[t][:], jb[:])
            jbk = npool.tile([64, S], FP32R)
            nc.sync.dma_start(jbk[0:32, :], kraw[32:64, :])
            nc.sync.dma_start(jbk[32:64, :], kraw[0:32, :])
            nc.vector.tensor_mul(kTr[:], kraw[:], cosq_sb[0:64, :])
            nc.vector.tensor_mul(jbk[:], jbk[:], sinq_sb[0:64, :])
            nc.vector.tensor_add(kTr[:], kTr[:], jbk[:])

            qodd = [qkpool.tile([64, S], FP32R, tag=f"qodd{t}", name=f"qodd{t}")
                    for t in range(2)]
            for t in range(2):
                nc.vector.tensor_copy(qodd[t][:], qTr[t][64:128, :])

            nc.sync.dma_start(kT_out[:], kTr[:])
            nc.sync.dma_start(vT_out[:], vT[:])

            # ---- phase 3: v65 (v back to [j, d] + ones column) ----
            ps_tp_cm = tc.tile_pool(name="ps_tp", bufs=2, space="PSUM")
            ps_tp = ps_tp_cm.__enter__()
            v65_all = qkpool.tile([128, NJ * 65], FP32R)
            for jt in range(NJ):
                tp = ps_tp.tile([128, 64], FP32, tag="tp", name=f"tp{jt}")
                nc.tensor.transpose(
                    tp[:], vT[:, jt * 128:(jt + 1) * 128], idn_sb[:]
                )
                nc.vector.tensor_copy(v65_all[:, jt * 65:jt * 65 + 64], tp[:])
                nc.vector.tensor_copy(v65_all[:, jt * 65 + 64:jt * 65 + 65],
                                      ones_sb[:])
            ps_tp_cm.__exit__(None, None, None)

            # ---- phase 4: attention + normalize ----
            attn = qraw

            ps_sc_cm = tc.tile_pool(name="ps_sc", bufs=1, space="PSUM")
            ps_sc = ps_sc_cm.__enter__()
            ps_pv_cm = tc.tile_pool(name="ps_pv", bufs=1, space="PSUM")
            ps_pv = ps_pv_cm.__enter__()
            ps_bc_cm = tc.tile_pool(name="ps_bc", bufs=1, space="PSUM")
            ps_bc = ps_bc_cm.__enter__()
            ps_wo_cm = tc.tile_pool(name="ps_wo", bufs=2, space="PSUM")
            ps_wo = ps_wo_cm.__enter__()

            wo_sb = wpool.tile([128, 16 * QCOLS], FP32R, tag="wqslot",
                               name="wo_sb")
            for k in range(2):
                nc.sync.dma_start(wo_sb[:, k * 2048:(k + 1) * 2048],
                                  wo[k * 128:(k + 1) * 128, :])
            for ic in range(NIC):
                i0 = ic * 512
                jlo = max(0, i0 - 511) // 128
                jhi = (i0 + 511) // 128
                jts = list(range(jlo, jhi + 1))
                groups = [jts[g:g + 2] for g in range(0, len(jts), 2)]
                den_ic = npool.tile([1, 2048], FP32, tag="den_ic",
                                    name=f"den_{ic}", bufs=2)
                pv_pair = []
                for t in range(2):
                    pv = ps_pv.tile([65, 1024], FP32, tag="pv",
                                    name=f"pv{t}_{ic}")
                    pv_pair.append(pv)
                    for hh in range(2):
                        h = 2 * t + hh
                        if hh == 0:
                            qsrc = qTr[t][0:64, i0:i0 + 512]
                        else:
                            qsrc = qodd[t][:, i0:i0 + 512]
                        pvs = pv[:, hh * 512:(hh + 1) * 512]
                        nmm = 0
                        for grp in groups:
                            sc = ps_sc.tile([128, 1024], FP32, tag="sc",
                                            name=f"sc{h}_{ic}_{grp[0]}")
                            for gi, jt in enumerate(grp):
                                nc.tensor.matmul(
                                    sc[:, gi * 512:(gi + 1) * 512],
                                    kTr[:, jt * 128:(jt + 1) * 128], qsrc,
                                    start=True, stop=True,
                                )
                            p = ppool.tile([128, 1024], FP32R, tag="p",
                                           name=f"p{h}_{ic}_{grp[0]}")
                            nc.scalar.activation(
                                p[:, 0:len(grp) * 512], sc[:, 0:len(grp) * 512],
                                AF.Exp,
                            )
                            for gi, jt in enumerate(grp):
                                delta = i0 - jt * 128
                                psl = p[:, gi * 512:(gi + 1) * 512]
                                if delta <= 0:
                                    nc.gpsimd.affine_select(
                                        psl, psl, pattern=[[1, 512]],
                                        compare_op=mybir.AluOpType.is_ge,
                                        fill=0.0, base=delta,
                                        channel_multiplier=-1,
                                    )
                                else:
                                    nc.gpsimd.affine_select(
                                        psl, psl, pattern=[[-1, 512]],
                                        compare_op=mybir.AluOpType.is_ge,
                                        fill=0.0, base=511 - delta,
                                        channel_multiplier=1,
                                    )
                                nc.tensor.matmul(
                                    pvs, v65_all[:, jt * 65:(jt + 1) * 65], psl,
                                    start=(nmm == 0), stop=(nmm == len(jts) - 1),
                                )
                                nmm += 1
                    nc.vector.tensor_copy(
                        den_ic[0:1, t * 1024:(t + 1) * 1024], pv[64:65, :])
                # reciprocal across 32 lanes: [1, 2048] -> [32, 64] via a
                # DRAM bounce (linear DRAM makes the reshape well-defined)
                nc.sync.dma_start(den_dram[ic:ic + 1, :], den_ic[0:1, :])
                den128 = npool.tile([32, 64], FP32, tag="den128",
                                    name=f"den128_{ic}", bufs=2)
                nc.sync.dma_start(den128[:], den_dram[ic:ic + 1, :].rearrange(
                    "a (p f) -> (a p) f", p=32))
                nc.vector.tensor_scalar_add(den128[:], den128[:], es_sb[0:32, :])
                rec128 = npool.tile([32, 64], FP32R, tag="rec128",
                                    name=f"rec128_{ic}", bufs=2)
                with nc.allow_low_precision(reason="recip feeds bcast matmul"):
                    nc.vector.reciprocal(rec128[:], den128[:])
                # back to row-per-head [2, 2048]: row hh, col block t*512
                nc.sync.dma_start(
                    rec_dram[ic:ic + 1, :].rearrange("a (p f) -> (a p) f", p=32),
                    rec128[:])
                rec2 = npool.tile([2, 2048], FP32R, tag="rec2",
                                  name=f"rec2_{ic}", bufs=2)
                for h in range(4):
                    t, hh = h // 2, h % 2
                    nc.sync.dma_start(
                        rec2[hh:hh + 1, t * 512:(t + 1) * 512],
                        rec_dram[ic:ic + 1, h * 512:(h + 1) * 512],
                    )
                for t in range(2):
                    bc = ps_bc.tile([128, 512], FP32, tag="bc",
                                    name=f"bc{t}_{ic}")
                    nc.tensor.matmul(bc[:], ind2_sb[:],
                                     rec2[:, t * 512:(t + 1) * 512],
                                     start=True, stop=True)
                    araw = npool.tile([128, 512], FP32, tag="araw",
                                      name=f"araw{t}_{ic}", bufs=2)
                    nc.scalar.copy(araw[0:64, :], pv_pair[t][0:64, 0:512])
                    nc.vector.tensor_copy(araw[64:128, :],
                                          pv_pair[t][0:64, 512:1024])
                    nc.vector.tensor_mul(attn[t][:, i0:i0 + 512], araw[:],
                                         bc[:])
                # w_o partial for this i-chunk's four s-tiles (keeps PE dense)
                for st in range(4 * ic, 4 * ic + 4):
                    s0 = st * 128
                    for half in range(2):
                        osb = opool.tile([128, 1024], FP32, tag="osb",
                                         name=f"osb{st}_{half}")
                        for nn in range(2):
                            n = half * 2 + nn
                            woo = ps_wo.tile([128, 512], FP32, tag="woo",
                                             name=f"woo{st}_{n}")
                            for k in range(2):
                                nc.tensor.matmul(
                                    woo[:],
                                    attn[k][:, s0:s0 + 128],
                                    wo_sb[:, k * 2048 + n * 512:
                                          k * 2048 + (n + 1) * 512],
                                    start=(k == 0), stop=(k == 1),
                                )
                            if n % 2 == 0:
                                nc.scalar.copy(
                                    osb[:, nn * 512:(nn + 1) * 512], woo[:])
                            else:
                                nc.vector.tensor_copy(
                                    osb[:, nn * 512:(nn + 1) * 512], woo[:])
                        nc.sync.dma_start(
                            out_part[s0:s0 + 128,
                                     half * 1024:(half + 1) * 1024],
                            osb[:])
            ps_wo_cm.__exit__(None, None, None)
            ps_bc_cm.__exit__(None, None, None)
            ps_pv_cm.__exit__(None, None, None)
            ps_sc_cm.__exit__(None, None, None)

    nc.compile()
    return nc


def _get_compiled():
    global _COMPILED
    if _COMPILED is None:
        _COMPILED = _build_kernel()
    return _COMPILED


def _host_inputs(positions, hidden_states, w_q, w_k, w_v, w_o, sink):
    positions = np.asarray(positions)
    hidden_states = np.asarray(hidden_states, dtype=np.float32)
    w_q = np.asarray(w_q, dtype=np.float32)
    w_k = np.asarray(w_k, dtype=np.float32)
    w_v = np.asarray(w_v, dtype=np.float32)
    w_o = np.asarray(w_o, dtype=np.float32)
    sink = np.asarray(sink, dtype=np.float32)

    hT = np.ascontiguousarray(hidden_states.T)

    half = D // 2
    inv_freq = 1.0 / (THETA ** (np.arange(half, dtype=np.float32) / half))
    ang = positions.astype(np.float32)[:, None] * inv_freq[None, :]
    cos = np.cos(ang).T.astype(np.float32)          # [32, S]
    sin = np.sin(ang).T.astype(np.float32)

    cos2 = np.concatenate([cos, cos], 0)            # [64, S]
    sin2s = np.concatenate([-sin, sin], 0)          # signed
    cosq_t = np.ascontiguousarray(np.concatenate([cos2, cos2], 0))
    sinq_t = np.ascontiguousarray(np.concatenate([sin2s, sin2s], 0))

    idn64 = np.eye(64, dtype=np.float32)
    ones_col = np.ones((128, 1), np.float32)
    ind2 = np.zeros((2, 128), np.float32)
    ind2[0, 0:64] = 1.0
    ind2[1, 64:128] = 1.0

    es = np.exp(sink).astype(np.float32)

    in_maps = []
    for c in range(NCORES):
        wq_c = np.ascontiguousarray(
            w_q[:, c * QCOLS:(c + 1) * QCOLS]) * np.float32(D ** -0.5)
        wkv_c = np.ascontiguousarray(
            np.concatenate([w_k[:, c * D:(c + 1) * D],
                            w_v[:, c * D:(c + 1) * D]], axis=1))
        wo_c = np.ascontiguousarray(w_o[c * QCOLS:(c + 1) * QCOLS, :])
        es_v = np.zeros((128, 1), np.float32)
        for h in range(HPC):
            es_v[h * 8:(h + 1) * 8, 0] = es[c * HPC + h]
        in_maps.append({
            "hT": hT, "wq": wq_c, "wkv": wkv_c, "wo": wo_c,
            "cosq": cosq_t, "sinq": sinq_t,
            "es_sh": es_v, "ones_col": ones_col,
            "ind2": ind2, "idn64": idn64,
        })
    return in_maps


def kernel(positions, hidden_states, w_q, w_k, w_v, w_o, sink, _res_holder=[]):
    from concourse.bass_utils import run_bass_kernel_spmd

    nc = _get_compiled()
    in_maps = _host_inputs(positions, hidden_states, w_q, w_k, w_v, w_o, sink)
    trace = os.environ.get("KERNEL_TRACE", "0") == "1"
    res = run_bass_kernel_spmd(nc, in_maps, list(range(NCORES)), trace=trace)
    _res_holder.clear()
    _res_holder.append(res)

    output = np.zeros((S, HID), np.float32)
    k_full = np.zeros((S, KVH * D), np.float32)
    v_full = np.zeros((S, KVH * D), np.float32)
    for c in range(NCORES):
        r = res.results[c]
        output += r["out_part"]
        k_full[:, c * D:(c + 1) * D] = r["kT_out"].T
        v_full[:, c * D:(c + 1) * D] = r["vT_out"].T
    kv_fused = np.concatenate([k_full, v_full], axis=1)
    return output, kv_fused


# revision 15
# speedup vs baseline: 1.4361x; 1.0636x over previous
"""MiMoV2 sliding-window attention (S=2048, H=32, KVH=8, D=64, WIN=512) on 8
Trainium2 NeuronCores, tensor-parallel across heads (4 q heads / 1 kv head per
core), host-side reduce of the w_o partial sums.

Per-core device kernel:
  hiddenT [HID, S] streamed as fp32r k-tiles; qT/kT/vT produced in transposed
  [head_dim, seq] form directly by the projection matmuls (D^-0.5 folded into
  w_q on the host); neox rope applied via SBUF->SBUF half-swap DMAs plus
  sign-folded trig tables; scoresT [j, i] = kT.T @ qT per 128-key j-tile; exp
  on ACT (psum->sbuf, two j-tiles per call); sliding-window mask via one
  gpsimd affine_select per tile; PV with an appended ones-column producing the
  softmax denominator for free; denominators for one i-chunk are gathered,
  DMA-reshaped to [32, 64] so the (slow, per-lane) DVE reciprocal runs once
  across 32 lanes, DMA'd back row-per-head, and broadcast across partitions
  with a K=2 indicator matmul straight into PSUM for the normalize multiply;
  w_o partial per s-tile (weights loaded into the released w_q slot), summed
  across cores on the host.
"""
import os
import numpy as np

S = 2048
HID = 2048
H = 32
KVH = 8
D = 64
WIN = 512
THETA = 1e6
NCORES = 8
HPC = H // NCORES        # q heads per core = 4
QCOLS = HPC * D          # 256

_COMPILED = None


def _build_kernel():
    import concourse.bass as bass
    import concourse.tile as tile
    from concourse import bacc, mybir

    FP32 = mybir.dt.float32
    FP32R = mybir.dt.float32r
    AF = mybir.ActivationFunctionType

    nc = bacc.Bacc("TRN2", target_bir_lowering=False, debug=False,
                   num_devices=NCORES)

    hT = nc.dram_tensor("hT", [HID, S], FP32R, kind="ExternalInput")
    wq = nc.dram_tensor("wq", [HID, QCOLS], FP32R, kind="ExternalInput")
    wkv = nc.dram_tensor("wkv", [HID, 128], FP32R, kind="ExternalInput")
    wo = nc.dram_tensor("wo", [QCOLS, HID], FP32R, kind="ExternalInput")
    cosq = nc.dram_tensor("cosq", [128, S], FP32R, kind="ExternalInput")
    sinq = nc.dram_tensor("sinq", [128, S], FP32R, kind="ExternalInput")
    es_sh = nc.dram_tensor("es_sh", [128, 1], FP32, kind="ExternalInput")
    ones_col = nc.dram_tensor("ones_col", [128, 1], FP32R, kind="ExternalInput")
    ind2 = nc.dram_tensor("ind2", [2, 128], FP32R, kind="ExternalInput")
    idn64 = nc.dram_tensor("idn64", [64, 64], FP32, kind="ExternalInput")

    den_dram = nc.dram_tensor("den_dram", [4, 2048], FP32,
                              kind="ExternalOutput")
    rec_dram = nc.dram_tensor("rec_dram", [4, 2048], FP32R,
                              kind="ExternalOutput")
    out_part = nc.dram_tensor("out_part", [S, HID], FP32, kind="ExternalOutput")
    kT_out = nc.dram_tensor("kT_out", [64, S], FP32R, kind="ExternalOutput")
    vT_out = nc.dram_tensor("vT_out", [64, S], FP32, kind="ExternalOutput")

    NJ = S // 128
    NIC = S // 512

    with tile.TileContext(nc) as tc:
        with (
            tc.tile_pool(name="const", bufs=1) as cpool,
            tc.tile_pool(name="wpool", bufs=1) as wpool,
            tc.tile_pool(name="hpool", bufs=4) as hpool,
            tc.tile_pool(name="qk", bufs=1) as qkpool,
            tc.tile_pool(name="ppool", bufs=3) as ppool,
            tc.tile_pool(name="npool", bufs=1) as npool,
            tc.tile_pool(name="opool", bufs=2) as opool,
        ):
            # ---- weights first (per-k contiguous chunks so the first
            # projection matmuls start as soon as chunk 0 lands) ----
            wq_sb = wpool.tile([128, 16 * QCOLS], FP32R, tag="wqslot",
                               name="wq_sb")
            wkv_sb = wpool.tile([128, 16 * 128], FP32R)

            # constants (needed from rope onward; scheduled behind phase 1)
            cosq_sb = cpool.tile([128, S], FP32R)
            nc.sync.dma_start(cosq_sb[:], cosq[:])
            sinq_sb = cpool.tile([128, S], FP32R)
            nc.sync.dma_start(sinq_sb[:], sinq[:])
            es_sb = cpool.tile([128, 1], FP32)
            nc.sync.dma_start(es_sb[:], es_sh[:])
            ones_sb = cpool.tile([128, 1], FP32R)
            nc.sync.dma_start(ones_sb[:], ones_col[:])
            ind2_sb = cpool.tile([2, 128], FP32R)
            nc.sync.dma_start(ind2_sb[:], ind2[:])
            idn_sb = cpool.tile([64, 64], FP32)
            nc.sync.dma_start(idn_sb[:], idn64[:])

            # ---- phase 1: projections ----
            qraw = [qkpool.tile([128, S], FP32R, tag=f"qraw{t}", name=f"qraw{t}")
                    for t in range(2)]
            kraw = qkpool.tile([64, S], FP32R)
            vT = qkpool.tile([64, S], FP32)
            qTr = [qkpool.tile([128, S], FP32R, tag=f"qTr{t}", name=f"qTr{t}")
                   for t in range(2)]
            kTr = qkpool.tile([64, S], FP32R)
            qodd = [qkpool.tile([64, S], FP32R, tag=f"qodd{t}", name=f"qodd{t}")
                    for t in range(2)]
            v65_all = qkpool.tile([128, NJ * 65], FP32R)

            ps_proj_cm = tc.tile_pool(name="ps_proj", bufs=1, space="PSUM")
            ps_proj = ps_proj_cm.__enter__()
            ps_tp_cm = tc.tile_pool(name="ps_tp", bufs=2, space="PSUM")
            ps_tp = ps_tp_cm.__enter__()
            for sh in range(2):
                sl = slice(sh * 1024, (sh + 1) * 1024)
                q_ps = [ps_proj.tile([128, 1024], FP32, tag=f"q_ps{m}",
                                     name=f"q_ps{m}_{sh}") for m in range(2)]
                kv_ps = ps_proj.tile([128, 1024], FP32, tag="kv_ps",
                                     name=f"kv_ps_{sh}")
                for k in range(16):
                    if sh == 0:
                        nc.sync.dma_start(wq_sb[:, k * QCOLS:(k + 1) * QCOLS],
                                          wq[k * 128:(k + 1) * 128, :])
                        nc.sync.dma_start(wkv_sb[:, k * 128:(k + 1) * 128],
                                          wkv[k * 128:(k + 1) * 128, :])
                    ht = hpool.tile([128, 1024], FP32R, tag="ht", name=f"ht{sh}_{k}")
                    nc.sync.dma_start(ht[:], hT[k * 128:(k + 1) * 128, sl])
                    st, sp = (k == 0), (k == 15)
                    for m in range(2):
                        for n in range(2):
                            nc.tensor.matmul(
                                q_ps[m][:, n * 512:(n + 1) * 512],
                                wq_sb[:, k * QCOLS + m * 128:k * QCOLS + (m + 1) * 128],
                                ht[:, n * 512:(n + 1) * 512],
                                start=st, stop=sp,
                            )
                    for n in range(2):
                        nc.tensor.matmul(
                            kv_ps[:, n * 512:(n + 1) * 512],
                            wkv_sb[:, k * 128:(k + 1) * 128],
                            ht[:, n * 512:(n + 1) * 512],
                            start=st, stop=sp,
                        )
                for m in range(2):
                    nc.scalar.copy(qraw[m][0:64, sl], q_ps[m][0:64, :])
                    nc.vector.tensor_copy(qraw[m][64:128, sl], q_ps[m][64:128, :])
                nc.scalar.copy(kraw[0:64, sl], kv_ps[0:64, :])
                nc.vector.tensor_copy(vT[0:64, sl], kv_ps[64:128, :])

                # rope + per-head extraction + v65 for this s-half
                for t in range(2):
                    jb = npool.tile([128, 1024], FP32R, tag="jbuf",
                                    name=f"jbuf{t}_{sh}", bufs=2)
                    for blk in range(2):
                        b0 = blk * 64
                        nc.sync.dma_start(jb[b0:b0 + 32, :],
                                          qraw[t][b0 + 32:b0 + 64, sl])
                        nc.sync.dma_start(jb[b0 + 32:b0 + 64, :],
                                          qraw[t][b0:b0 + 32, sl])
                    nc.vector.tensor_mul(qTr[t][:, sl], qraw[t][:, sl],
                                         cosq_sb[:, sl])
                    nc.vector.tensor_mul(jb[:], jb[:], sinq_sb[:, sl])
                    nc.vector.tensor_add(qTr[t][:, sl], qTr[t][:, sl], jb[:])
                    nc.vector.tensor_copy(qodd[t][:, sl], qTr[t][64:128, sl])
                jbk = npool.tile([64, 1024], FP32R, tag="jbk",
                                 name=f"jbk{sh}", bufs=2)
                nc.sync.dma_start(jbk[0:32, :], kraw[32:64, sl])
                nc.sync.dma_start(jbk[32:64, :], kraw[0:32, sl])
                nc.vector.tensor_mul(kTr[:, sl], kraw[:, sl], cosq_sb[0:64, sl])
                nc.vector.tensor_mul(jbk[:], jbk[:], sinq_sb[0:64, sl])
                nc.vector.tensor_add(kTr[:, sl], kTr[:, sl], jbk[:])

                for jt in range(8 * sh, 8 * sh + 8):
                    tp = ps_tp.tile([128, 64], FP32, tag="tp", name=f"tp{jt}")
                    nc.tensor.transpose(
                        tp[:], vT[:, jt * 128:(jt + 1) * 128], idn_sb[:]
                    )
                    nc.vector.tensor_copy(v65_all[:, jt * 65:jt * 65 + 64],
                                          tp[:])
                    nc.vector.tensor_copy(
                        v65_all[:, jt * 65 + 64:jt * 65 + 65], ones_sb[:])

                nc.sync.dma_start(kT_out[:, sl], kTr[:, sl])
                nc.sync.dma_start(vT_out[:, sl], vT[:, sl])
            ps_tp_cm.__exit__(None, None, None)
            ps_proj_cm.__exit__(None, None, None)

            # ---- phase 4: attention + normalize ----
            attn = qraw

            ps_sc_cm = tc.tile_pool(name="ps_sc", bufs=2, space="PSUM")
            ps_sc = ps_sc_cm.__enter__()
            ps_pv_cm = tc.tile_pool(name="ps_pv", bufs=1, space="PSUM")
            ps_pv = ps_pv_cm.__enter__()
            ps_bc_cm = tc.tile_pool(name="ps_bc", bufs=1, space="PSUM")
            ps_bc = ps_bc_cm.__enter__()
            ps_wo_cm = tc.tile_pool(name="ps_wo", bufs=1, space="PSUM")
            ps_wo = ps_wo_cm.__enter__()

            wo_sb = wpool.tile([128, 16 * QCOLS], FP32R, tag="wqslot",
                               name="wo_sb")
            for k in range(2):
                nc.sync.dma_start(wo_sb[:, k * 2048:(k + 1) * 2048],
                                  wo[k * 128:(k + 1) * 128, :])
            for ic in range(NIC):
                i0 = ic * 512
                jlo = max(0, i0 - 511) // 128
                jhi = (i0 + 511) // 128
                jts = list(range(jlo, jhi + 1))
                groups = [jts[g:g + 2] for g in range(0, len(jts), 2)]
                den_ic = npool.tile([1, 2048], FP32, tag="den_ic",
                                    name=f"den_{ic}", bufs=2)
                pv_pair = []
                for t in range(2):
                    pv = ps_pv.tile([65, 1024], FP32, tag="pv",
                                    name=f"pv{t}_{ic}")
                    pv_pair.append(pv)
                    for hh in range(2):
                        h = 2 * t + hh
                        if hh == 0:
                            qsrc = qTr[t][0:64, i0:i0 + 512]
                        else:
                            qsrc = qodd[t][:, i0:i0 + 512]
                        pvs = pv[:, hh * 512:(hh + 1) * 512]
                        nmm = 0
                        for grp in groups:
                            sc = ps_sc.tile([128, 1024], FP32, tag="sc",
                                            name=f"sc{h}_{ic}_{grp[0]}")
                            for gi, jt in enumerate(grp):
                                nc.tensor.matmul(
                                    sc[:, gi * 512:(gi + 1) * 512],
                                    kTr[:, jt * 128:(jt + 1) * 128], qsrc,
                                    start=True, stop=True,
                                )
                            p = ppool.tile([128, 1024], FP32R, tag="p",
                                           name=f"p{h}_{ic}_{grp[0]}")
                            nc.scalar.activation(
                                p[:, 0:len(grp) * 512], sc[:, 0:len(grp) * 512],
                                AF.Exp,
                            )
                            d0 = i0 - grp[0] * 128
                            ng = len(grp)
                            psel = p[:, 0:ng * 512].rearrange(
                                "q (g i) -> q g i", g=ng)
                            if d0 <= 0:
                                # causal: keep  d0 - 128*g - q + i >= 0
                                nc.gpsimd.affine_select(
                                    psel, psel, pattern=[[-128, ng], [1, 512]],
                                    compare_op=mybir.AluOpType.is_ge,
                                    fill=0.0, base=d0, channel_multiplier=-1,
                                )
                            else:
                                # window: keep (511-d0) + 128*g + q - i >= 0
                                nc.gpsimd.affine_select(
                                    psel, psel, pattern=[[128, ng], [-1, 512]],
                                    compare_op=mybir.AluOpType.is_ge,
                                    fill=0.0, base=511 - d0,
                                    channel_multiplier=1,
                                )
                            for gi, jt in enumerate(grp):
                                psl = p[:, gi * 512:(gi + 1) * 512]
                                nc.tensor.matmul(
                                    pvs, v65_all[:, jt * 65:(jt + 1) * 65], psl,
                                    start=(nmm == 0), stop=(nmm == len(jts) - 1),
                                )
                                nmm += 1
                    nc.vector.tensor_copy(
                        den_ic[0:1, t * 1024:(t + 1) * 1024], pv[64:65, :])
                # reciprocal across 32 lanes: [1, 2048] -> [32, 64] via a
                # DRAM bounce (linear DRAM makes the reshape well-defined)
                nc.sync.dma_start(den_dram[ic:ic + 1, :], den_ic[0:1, :])
                den128 = npool.tile([32, 64], FP32, tag="den128",
                                    name=f"den128_{ic}", bufs=2)
                nc.sync.dma_start(den128[:], den_dram[ic:ic + 1, :].rearrange(
                    "a (p f) -> (a p) f", p=32))
                nc.vector.tensor_scalar_add(den128[:], den128[:], es_sb[0:32, :])
                rec128 = npool.tile([32, 64], FP32R, tag="rec128",
                                    name=f"rec128_{ic}", bufs=2)
                with nc.allow_low_precision(reason="recip feeds bcast matmul"):
                    nc.vector.reciprocal(rec128[:], den128[:])
                # back to row-per-head [2, 2048]: row hh, col block t*512
                nc.sync.dma_start(
                    rec_dram[ic:ic + 1, :].rearrange("a (p f) -> (a p) f", p=32),
                    rec128[:])
                rec2 = npool.tile([2, 2048], FP32R, tag="rec2",
                                  name=f"rec2_{ic}", bufs=2)
                for h in range(4):
                    t, hh = h // 2, h % 2
                    nc.sync.dma_start(
                        rec2[hh:hh + 1, t * 512:(t + 1) * 512],
                        rec_dram[ic:ic + 1, h * 512:(h + 1) * 512],
                    )
                for t in range(2):
                    bc = ps_bc.tile([128, 512], FP32, tag="bc",
                                    name=f"bc{t}_{ic}")
                    nc.tensor.matmul(bc[:], ind2_sb[:],
                                     rec2[:, t * 512:(t + 1) * 512],
                                     start=True, stop=True)
                    araw = npool.tile([128, 512], FP32, tag="araw",
                                      name=f"araw{t}_{ic}", bufs=2)
                    nc.scalar.copy(araw[0:64, :], pv_pair[t][0:64, 0:512])
                    nc.vector.tensor_copy(araw[64:128, :],
                                          pv_pair[t][0:64, 512:1024])
                    nc.vector.tensor_mul(attn[t][:, i0:i0 + 512], araw[:],
                                         bc[:])
                # w_o partial for this i-chunk's four s-tiles (keeps PE dense)
                for st in range(4 * ic, 4 * ic + 4):
                    s0 = st * 128
                    for half in range(2):
                        osb = opool.tile([128, 1024], FP32, tag="osb",
                                         name=f"osb{st}_{half}")
                        for nn in range(2):
                            n = half * 2 + nn
                            woo = ps_wo.tile([128, 512], FP32, tag="woo",
                                             name=f"woo{st}_{n}")
                            for k in range(2):
                                nc.tensor.matmul(
                                    woo[:],
                                    attn[k][:, s0:s0 + 128],
                                    wo_sb[:, k * 2048 + n * 512:
                                          k * 2048 + (n + 1) * 512],
                                    start=(k == 0), stop=(k == 1),
                                )
                            if n % 2 == 0:
                                nc.scalar.copy(
                                    osb[:, nn * 512:(nn + 1) * 512], woo[:])
                            else:
                                nc.vector.tensor_copy(
                                    osb[:, nn * 512:(nn + 1) * 512], woo[:])
                        nc.sync.dma_start(
                            out_part[s0:s0 + 128,
                                     half * 1024:(half + 1) * 1024],
                            osb[:])
            ps_wo_cm.__exit__(None, None, None)
            ps_bc_cm.__exit__(None, None, None)
            ps_pv_cm.__exit__(None, None, None)
            ps_sc_cm.__exit__(None, None, None)

    nc.compile()
    return nc


def _get_compiled():
    global _COMPILED
    if _COMPILED is None:
        _COMPILED = _build_kernel()
    return _COMPILED


def _host_inputs(positions, hidden_states, w_q, w_k, w_v, w_o, sink):
    positions = np.asarray(positions)
    hidden_states = np.asarray(hidden_states, dtype=np.float32)
    w_q = np.asarray(w_q, dtype=np.float32)
    w_k = np.asarray(w_k, dtype=np.float32)
    w_v = np.asarray(w_v, dtype=np.float32)
    w_o = np.asarray(w_o, dtype=np.float32)
    sink = np.asarray(sink, dtype=np.float32)

    hT = np.ascontiguousarray(hidden_states.T)

    half = D // 2
    inv_freq = 1.0 / (THETA ** (np.arange(half, dtype=np.float32) / half))
    ang = positions.astype(np.float32)[:, None] * inv_freq[None, :]
    cos = np.cos(ang).T.astype(np.float32)          # [32, S]
    sin = np.sin(ang).T.astype(np.float32)

    cos2 = np.concatenate([cos, cos], 0)            # [64, S]
    sin2s = np.concatenate([-sin, sin], 0)          # signed
    cosq_t = np.ascontiguousarray(np.concatenate([cos2, cos2], 0))
    sinq_t = np.ascontiguousarray(np.concatenate([sin2s, sin2s], 0))

    idn64 = np.eye(64, dtype=np.float32)
    ones_col = np.ones((128, 1), np.float32)
    ind2 = np.zeros((2, 128), np.float32)
    ind2[0, 0:64] = 1.0
    ind2[1, 64:128] = 1.0

    es = np.exp(sink).astype(np.float32)

    in_maps = []
    for c in range(NCORES):
        wq_c = np.ascontiguousarray(
            w_q[:, c * QCOLS:(c + 1) * QCOLS]) * np.float32(D ** -0.5)
        wkv_c = np.ascontiguousarray(
            np.concatenate([w_k[:, c * D:(c + 1) * D],
                            w_v[:, c * D:(c + 1) * D]], axis=1))
        wo_c = np.ascontiguousarray(w_o[c * QCOLS:(c + 1) * QCOLS, :])
        es_v = np.zeros((128, 1), np.float32)
        for h in range(HPC):
            es_v[h * 8:(h + 1) * 8, 0] = es[c * HPC + h]
        in_maps.append({
            "hT": hT, "wq": wq_c, "wkv": wkv_c, "wo": wo_c,
            "cosq": cosq_t, "sinq": sinq_t,
            "es_sh": es_v, "ones_col": ones_col,
            "ind2": ind2, "idn64": idn64,
        })
    return in_maps


def kernel(positions, hidden_states, w_q, w_k, w_v, w_o, sink, _res_holder=[]):
    from concourse.bass_utils import run_bass_kernel_spmd

    nc = _get_compiled()
    in_maps = _host_inputs(positions, hidden_states, w_q, w_k, w_v, w_o, sink)
    trace = os.environ.get("KERNEL_TRACE", "0") == "1"
    res = run_bass_kernel_spmd(nc, in_maps, list(range(NCORES)), trace=trace)
    _res_holder.clear()
    _res_holder.append(res)

    output = np.zeros((S, HID), np.float32)
    k_full = np.zeros((S, KVH * D), np.float32)
    v_full = np.zeros((S, KVH * D), np.float32)
    for c in range(NCORES):
        r = res.results[c]
        output += r["out_part"]
        k_full[:, c * D:(c + 1) * D] = r["kT_out"].T
        v_full[:, c * D:(c + 1) * D] = r["vT_out"].T
    kv_fused = np.concatenate([k_full, v_full], axis=1)
    return output, kv_fused
